# revision 1
# baseline (speedup 1.0000x reference)
import sys

sys.path.insert(0, "/opt/trn_rl_repo")

from contextlib import ExitStack

import numpy as np
import ml_dtypes
import concourse.bacc as bacc
import concourse.bass as bass
import concourse.mybir as mybir
from concourse.bass_utils import run_bass_kernel_spmd
from concourse.tile import TileContext
from concourse.library_config import mlp as _mlp_lib
from concourse.masks import make_identity

P = 128
NCORES = 8
N, D, E, KHOP, B, L = 100000, 128, 1600000, 3, 32768, 262144
H_MLP, R = 512, 64
NBLK = 98            # dst blocks per core (uniform; core 7 pads)
SHARD = NBLK * P     # 12544 rows per core slot
NP = NCORES * SHARD  # 100352 padded rows
QROWS = 32768
NQ = 4               # src quadrants (int16 gather index range)
OCT = 4              # dst blocks per gather call
BSEG = B // NCORES   # 4096 segments per core
NSB = BSEG // P      # 32 segment blocks per core
POCT = 4             # segment blocks per pool gather call

f32 = mybir.dt.float32
bf16 = mybir.dt.bfloat16
i16 = mybir.dt.int16
i32 = mybir.dt.int32

_COMPILED = {}


def _wrap_idx16(idx):
    """dma_gather index layout: token i -> partition i%16, col i//16, x8 replicated."""
    n = len(idx)
    assert n % 16 == 0
    return np.tile(idx.reshape(n // 16, 16).T.astype(np.int16), (8, 1))


def _schedule2(blk_by_core, loc_by_core, src_by_core, nblk, octsz):
    """Like _schedule but fills loc values properly."""
    ncores = len(blk_by_core)
    counts = np.zeros((ncores, nblk, NQ), np.int64)
    for c in range(ncores):
        q = src_by_core[c] // QROWS
        np.add.at(counts[c], (blk_by_core[c], q), 1)
    gsz = ((counts.max(axis=0) + 127) // 128) * 128
    goff2d = np.zeros((nblk, NQ), np.int64)
    pos = 0
    noct = (nblk + octsz - 1) // octsz
    for o in range(noct):
        for q in range(NQ):
            for b in range(o * octsz, min((o + 1) * octsz, nblk)):
                goff2d[b, q] = pos
                pos += int(gsz[b, q])
    tok = pos
    srcs, locs = [], []
    for c in range(ncores):
        blk = blk_by_core[c].astype(np.int64)
        q = (src_by_core[c] // QROWS).astype(np.int64)
        order = np.lexsort((q, blk))
        bs, qs = blk[order], q[order]
        sl = (src_by_core[c][order] - qs * QROWS).astype(np.int16)
        dl = loc_by_core[c][order].astype(np.float32)
        key = bs * NQ + qs
        run_start = np.concatenate([[0], np.cumsum(counts[c].reshape(-1))])
        pos_in_run = np.arange(len(key)) - run_start[key]
        out_pos = goff2d.reshape(-1)[key] + pos_in_run
        src_full = np.zeros(tok, np.int16)
        loc_full = np.full(tok, -1.0, np.float32)
        src_full[out_pos] = sl
        loc_full[out_pos] = dl
        srcs.append(src_full)
        locs.append(loc_full)
    return gsz, goff2d, tok, srcs, locs


def _build_program(gszH, goffH, tokH, gszPh, goffPh, tokPh, gszPt, goffPt, tokPt):
    nc = bacc.Bacc("TRN2", target_bir_lowering=False, num_devices=NCORES)

    CHH, CHPh, CHPt = tokH // P, tokPh // P, tokPt // P

    embed_sh = nc.dram_tensor("embed_sh", [SHARD, D], f32, kind="ExternalInput")
    temp_in = nc.dram_tensor("temp_in", [P, 4], f32, kind="ExternalInput")
    wrep_in = nc.dram_tensor("wrep_in", [P, D], f32, kind="ExternalInput")
    w1_in = nc.dram_tensor("w1_in", [3, P, H_MLP], bf16, kind="ExternalInput")
    b1_in = nc.dram_tensor("b1_in", [P, 4], f32, kind="ExternalInput")
    w2_in = nc.dram_tensor("w2_in", [4, P, R], bf16, kind="ExternalInput")
    b2_in = nc.dram_tensor("b2_in", [R, 1], f32, kind="ExternalInput")
    dpl = nc.dram_tensor("dpl", [P, NBLK], i32, kind="ExternalInput")
    dph = nc.dram_tensor("dph", [P, NBLK], i32, kind="ExternalInput")
    spl = nc.dram_tensor("spl", [P, NBLK], i32, kind="ExternalInput")
    sph = nc.dram_tensor("sph", [P, NBLK], i32, kind="ExternalInput")
    hsrc = nc.dram_tensor("hsrc", [P, tokH // 16], i16, kind="ExternalInput")
    hloc = nc.dram_tensor("hloc", [P, CHH], f32, kind="ExternalInput")
    psrcH = nc.dram_tensor("psrcH", [P, tokPh // 16], i16, kind="ExternalInput")
    plocH = nc.dram_tensor("plocH", [P, CHPh], f32, kind="ExternalInput")
    psrcT = nc.dram_tensor("psrcT", [P, tokPt // 16], i16, kind="ExternalInput")
    plocT = nc.dram_tensor("plocT", [P, CHPt], f32, kind="ExternalInput")

    out = nc.dram_tensor("out", [BSEG, R], f32, kind="ExternalOutput")

    shard_a = nc.dram_tensor("shard_a", [SHARD, D], bf16)
    shard_b = nc.dram_tensor("shard_b", [SHARD, D], bf16)
    xab_a = nc.dram_tensor("xab_a", [NP, D], bf16, addr_space="Shared")
    xab_b = nc.dram_tensor("xab_b", [NP, D], bf16, addr_space="Shared")
    zsh = nc.dram_tensor("zsh", [SHARD, 2 * D], bf16)
    zfull = nc.dram_tensor("zfull", [NP, 2 * D], bf16, addr_space="Shared")

    rg = [list(range(NCORES))]

    with TileContext(nc) as tc, ExitStack() as ctx:
        sb = ctx.enter_context(tc.tile_pool(name="sb", bufs=2))
        const = ctx.enter_context(tc.tile_pool(name="const", bufs=1))
        gpool = ctx.enter_context(tc.tile_pool(name="gath", bufs=6))
        ohp = ctx.enter_context(tc.tile_pool(name="ohp", bufs=4))
        ccs = ctx.enter_context(nc.semaphore("ccs"))
        ccs_val = [0]

        def ag(ins_ap, outs_ap):
            tc.strict_bb_all_engine_barrier()
            with tc.tile_critical():
                ccs_val[0] += 1
                nc.gpsimd.collective_compute(
                    "AllGather", mybir.AluOpType.bypass,
                    ins=[ins_ap], outs=[outs_ap], replica_groups=rg,
                ).then_inc(ccs, 1)
                nc.gpsimd.wait_ge(ccs, ccs_val[0])
            tc.strict_bb_all_engine_barrier()

        nc.gpsimd.load_library(_mlp_lib)

        # ---------- constants ----------
        iota_i = const.tile([P, P], i32)
        nc.gpsimd.iota(iota_i[:], pattern=[[1, P]], base=0, channel_multiplier=0)
        iota_b = const.tile([P, P], bf16)
        nc.vector.tensor_copy(iota_b[:], iota_i[:])
        ident = const.tile([P, P], f32)
        make_identity(nc, ident[:])
        ones_col = const.tile([P, 1], bf16)
        nc.vector.memset(ones_col[:], 1.0)
        ones_row1 = const.tile([1, P], f32)
        nc.vector.memset(ones_row1[:], 1.0)

        temp_sb = const.tile([P, 4], f32)
        nc.sync.dma_start(temp_sb[:], temp_in[:])
        wrep = const.tile([P, D], f32)
        nc.sync.dma_start(wrep[:], wrep_in[:])
        w1t = const.tile([P, 3, H_MLP], bf16)
        nc.sync.dma_start(w1t[:], w1_in.rearrange("k p h -> p k h")[:])
        b1t = const.tile([P, 4], f32)
        nc.sync.dma_start(b1t[:], b1_in[:])
        w2t = const.tile([P, 4, R], bf16)
        nc.sync.dma_start(w2t[:], w2_in.rearrange("k p r -> p k r")[:])
        b2t = const.tile([R, 1], f32)
        nc.sync.dma_start(b2t[:], b2_in[:])

        # ---------- degree scales (from CSR rowptrs) ----------
        def rsqrt_deg(lo_ap, hi_ap, tag):
            lo_t = sb.tile([P, NBLK], i32, tag="degi")
            hi_t = sb.tile([P, NBLK], i32, tag="degi2")
            nc.sync.dma_start(lo_t[:], lo_ap)
            nc.sync.dma_start(hi_t[:], hi_ap)
            lo_f = sb.tile([P, NBLK], f32, tag="degf")
            hi_f = sb.tile([P, NBLK], f32, tag="degf2")
            nc.vector.tensor_copy(lo_f[:], lo_t[:])
            nc.vector.tensor_copy(hi_f[:], hi_t[:])
            deg = sb.tile([P, NBLK], f32, tag="deg0")
            nc.vector.tensor_tensor(out=deg[:], in0=hi_f[:], in1=lo_f[:],
                                    op=mybir.AluOpType.subtract)
            degm = sb.tile([P, NBLK], f32, tag="deg1")
            nc.vector.tensor_scalar(out=degm[:], in0=deg[:], scalar1=1.0,
                                    scalar2=None, op0=mybir.AluOpType.max)
            reci = sb.tile([P, NBLK], f32, tag="deg2")
            nc.vector.reciprocal(reci[:], degm[:])
            res = const.tile([P, NBLK], f32, tag=tag)
            nc.scalar.activation(res[:], reci[:],
                                 mybir.ActivationFunctionType.Sqrt)
            return res

        b_sc = rsqrt_deg(dpl[:], dph[:], "b_sc")
        a_sc = rsqrt_deg(spl[:], sph[:], "a_sc")
        ab_sc = const.tile([P, NBLK], f32)
        nc.vector.tensor_tensor(out=ab_sc[:], in0=a_sc[:], in1=b_sc[:],
                                op=mybir.AluOpType.mult)
        bt_sc = const.tile([P, 3, NBLK], f32)
        for k in range(3):
            nc.vector.tensor_scalar(out=bt_sc[:, k, :], in0=b_sc[:],
                                    scalar1=temp_sb[:, k + 1:k + 2], scalar2=None,
                                    op0=mybir.AluOpType.mult)

        # ---------- init ----------
        hidden = const.tile([P, NBLK, D], f32)
        nc.sync.dma_start(hidden[:], embed_sh.rearrange("(t p) d -> p t d", p=P)[:])
        for t in range(NBLK):
            g0 = sb.tile([P, D], bf16, tag="gout")
            nc.any.tensor_scalar(out=g0[:], in0=hidden[:, t, :],
                                 scalar1=a_sc[:, t:t + 1], scalar2=None,
                                 op0=mybir.AluOpType.mult)
            nc.sync.dma_start(
                shard_a.rearrange("(t p) d -> p t d", p=P)[:, t, :], g0[:])
        nc.vector.tensor_scalar(out=hidden[:], in0=hidden[:],
                                scalar1=temp_sb[:, 0:1], scalar2=None,
                                op0=mybir.AluOpType.mult)
        ag(shard_a[:], xab_a[:])

        # ---------- hops ----------
        hop_idx = const.tile([P, tokH // 16], i16)
        nc.sync.dma_start(hop_idx[:], hsrc[:])
        hop_loc = const.tile([P, CHH], f32)
        nc.sync.dma_start(hop_loc[:], hloc[:])

        noct = (NBLK + OCT - 1) // OCT
        with tc.tile_pool(name="psh", bufs=4, space="PSUM") as psh:
            for k in range(KHOP):
                src_full = xab_a if k % 2 == 0 else xab_b
                dst_shard = shard_b if k % 2 == 0 else shard_a
                dst_full = xab_b if k % 2 == 0 else xab_a
                for o in range(noct):
                    b0, b1 = o * OCT, min((o + 1) * OCT, NBLK)
                    tiles = {}
                    for q in range(NQ):
                        ntok = int(gszH[b0:b1, q].sum())
                        if ntok == 0:
                            continue
                        gt = gpool.tile([P, ntok // P, D], bf16, tag="gt")
                        tiles[q] = gt
                        t0 = int(goffH[b0, q])
                        for s0_ in range(0, ntok, 2048):
                            n_ = min(2048, ntok - s0_)
                            nc.gpsimd.dma_gather(
                                gt[:, s0_ // P:(s0_ + n_) // P, :],
                                src_full[q * QROWS:min((q + 1) * QROWS, NP), :],
                                hop_idx[:, (t0 + s0_) // 16:(t0 + s0_ + n_) // 16],
                                n_, n_, D, single_packet=False)
                    for b in range(b0, b1):
                        acc = psh.tile([P, D], f32, tag="acc")
                        nch_tot = int(gszH[b, :].sum()) // P
                        done = 0
                        for q in range(NQ):
                            g = int(gszH[b, q])
                            if g == 0:
                                continue
                            coff = int(gszH[b0:b, q].sum()) // P
                            c0 = int(goffH[b, q]) // P
                            for ch in range(g // P):
                                oh = ohp.tile([P, P], bf16, tag="oh")
                                nc.any.tensor_scalar(
                                    out=oh[:], in0=iota_b[:],
                                    scalar1=hop_loc[:, c0 + ch:c0 + ch + 1],
                                    scalar2=None, op0=mybir.AluOpType.is_equal)
                                nc.tensor.matmul(
                                    acc[:], lhsT=oh[:],
                                    rhs=tiles[q][:, coff + ch, :],
                                    start=done == 0, stop=done == nch_tot - 1)
                                done += 1
                        tmp = sb.tile([P, D], f32, tag="ep")
                        nc.any.tensor_scalar(out=tmp[:], in0=acc[:],
                                             scalar1=bt_sc[:, k, b:b + 1],
                                             scalar2=None,
                                             op0=mybir.AluOpType.mult)
                        nc.any.tensor_tensor(out=hidden[:, b, :],
                                             in0=hidden[:, b, :], in1=tmp[:],
                                             op=mybir.AluOpType.add)
                        if k < KHOP - 1:
                            gn = sb.tile([P, D], bf16, tag="gout")
                            nc.any.tensor_scalar(out=gn[:], in0=acc[:],
                                                 scalar1=ab_sc[:, b:b + 1],
                                                 scalar2=None,
                                                 op0=mybir.AluOpType.mult)
                            nc.sync.dma_start(
                                dst_shard.rearrange("(t p) d -> p t d", p=P)[:, b, :],
                                gn[:])
                if k < KHOP - 1:
                    ag(dst_shard[:], dst_full[:])

        # ---------- z_ext = [bf16(z) | s | 0] ----------
        for t in range(NBLK):
            ze = sb.tile([P, 2 * D], bf16, tag="ze")
            nc.vector.tensor_copy(ze[:, :D], hidden[:, t, :])
            prod = sb.tile([P, D], f32, tag="ep")
            nc.any.tensor_tensor(out=prod[:], in0=hidden[:, t, :], in1=wrep[:],
                                 op=mybir.AluOpType.mult)
            s_col = sb.tile([P, 1], f32, tag="scol")
            nc.vector.reduce_sum(s_col[:], prod[:], axis=mybir.AxisListType.X)
            nc.vector.tensor_copy(ze[:, D:D + 1], s_col[:])
            nc.vector.memset(ze[:, D + 1:], 0.0)
            nc.sync.dma_start(
                zsh.rearrange("(t p) d -> p t d", p=P)[:, t, :], ze[:])
        ag(zsh[:], zfull[:])

        # ---------- pooling ----------
        def pool(src16_t, loc_t, gszP, goffP, out_tag):
            pooled = const.tile([P, NSB, P], bf16, tag=out_tag)
            pnoct = (NSB + POCT - 1) // POCT
            with tc.tile_pool(name="psp" + out_tag, bufs=2, space="PSUM") as psp:
                for o in range(pnoct):
                    s0, s1 = o * POCT, min((o + 1) * POCT, NSB)
                    tiles = {}
                    for q in range(NQ):
                        ntok = int(gszP[s0:s1, q].sum())
                        if ntok == 0:
                            continue
                        gt = gpool.tile([P, ntok // P, 2 * D], bf16, tag="gt")
                        tiles[q] = gt
                        t0 = int(goffP[s0, q])
                        for s0_ in range(0, ntok, 1024):
                            n_ = min(1024, ntok - s0_)
                            nc.gpsimd.dma_gather(
                                gt[:, s0_ // P:(s0_ + n_) // P, :],
                                zfull[q * QROWS:min((q + 1) * QROWS, NP), :],
                                src16_t[:, (t0 + s0_) // 16:(t0 + s0_ + n_) // 16],
                                n_, n_, 2 * D, single_packet=False)
                    for sbk in range(s0, s1):
                        nch_tot = int(gszP[sbk, :].sum()) // P
                        if nch_tot == 0:
                            nc.vector.memset(pooled[:, sbk, :], 0.0)
                            continue
                        accf = psp.tile([P, P], f32, tag="accf")
                        accd = psp.tile([1, P], f32, tag="accd")
                        done = 0
                        for q in range(NQ):
                            g = int(gszP[sbk, q])
                            if g == 0:
                                continue
                            coff = int(gszP[s0:sbk, q].sum()) // P
                            c0 = int(goffP[sbk, q]) // P
                            for ch in range(g // P):
                                gch = tiles[q][:, coff + ch, :]
                                e_col = sb.tile([P, 1], f32, tag="ecol")
                                nc.scalar.activation(
                                    e_col[:], gch[:, D:D + 1],
                                    mybir.ActivationFunctionType.Exp)
                                oh = ohp.tile([P, P], bf16, tag="oh")
                                nc.any.tensor_scalar(
                                    out=oh[:], in0=iota_b[:],
                                    scalar1=loc_t[:, c0 + ch:c0 + ch + 1],
                                    scalar2=None, op0=mybir.AluOpType.is_equal)
                                ohw = ohp.tile([P, P], bf16, tag="ohw")
                                nc.any.tensor_scalar(
                                    out=ohw[:], in0=oh[:], scalar1=e_col[:],
                                    scalar2=None, op0=mybir.AluOpType.mult)
                                nc.tensor.matmul(accf[:], lhsT=gch[:, :D],
                                                 rhs=ohw[:], start=done == 0,
                                                 stop=done == nch_tot - 1)
                                nc.tensor.matmul(accd[:], lhsT=ones_col[:],
                                                 rhs=ohw[:], start=done == 0,
                                                 stop=done == nch_tot - 1)
                                done += 1
                        den = sb.tile([1, P], f32, tag="den")
                        nc.vector.tensor_scalar(out=den[:], in0=accd[:],
                                                scalar1=1e-30, scalar2=None,
                                                op0=mybir.AluOpType.max)
                        deni = sb.tile([1, P], f32, tag="deni")
                        nc.vector.reciprocal(deni[:], den[:])
                        rep = psp.tile([P, P], f32, tag="rep")
                        nc.tensor.matmul(rep[:], lhsT=ones_row1[:], rhs=deni[:],
                                         start=True, stop=True)
                        reps = sb.tile([P, P], f32, tag="reps")
                        nc.vector.tensor_copy(reps[:], rep[:])
                        nc.any.tensor_tensor(out=pooled[:, sbk, :], in0=accf[:],
                                             in1=reps[:],
                                             op=mybir.AluOpType.mult)
            return pooled

        pool_idx_h = const.tile([P, tokPh // 16], i16)
        nc.sync.dma_start(pool_idx_h[:], psrcH[:])
        pool_loc_h = const.tile([P, CHPh], f32)
        nc.sync.dma_start(pool_loc_h[:], plocH[:])
        pool_idx_t = const.tile([P, tokPt // 16], i16)
        nc.sync.dma_start(pool_idx_t[:], psrcT[:])
        pool_loc_t = const.tile([P, CHPt], f32)
        nc.sync.dma_start(pool_loc_t[:], plocT[:])

        hP = pool(pool_idx_h, pool_loc_h, gszPh, goffPh, "poolh")
        tP = pool(pool_idx_t, pool_loc_t, gszPt, goffPt, "poolt")
        htP = const.tile([P, NSB, P], bf16)
        nc.any.tensor_tensor(out=htP[:], in0=hP[:], in1=tP[:],
                             op=mybir.AluOpType.mult)
        feats = [hP, tP, htP]

        # ---------- MLP (feat-major) ----------
        with tc.tile_pool(name="psm", bufs=2, space="PSUM") as psm:
            for t in range(NSB):
                o1 = sb.tile([P, 4, P], bf16, tag="o1")
                for m in range(4):
                    ps1 = psm.tile([P, P], f32, tag="ps1")
                    for kk in range(3):
                        nc.tensor.matmul(ps1[:],
                                         lhsT=w1t[:, kk, m * P:(m + 1) * P],
                                         rhs=feats[kk][:, t, :],
                                         start=kk == 0, stop=kk == 2)
                    nc.scalar.activation(o1[:, m, :], ps1[:],
                                         mybir.ActivationFunctionType.Relu,
                                         bias=b1t[:, m:m + 1])
                ps2 = psm.tile([R, P], f32, tag="ps2")
                for kk in range(4):
                    nc.tensor.matmul(ps2[:], lhsT=w2t[:, kk, :], rhs=o1[:, kk, :],
                                     start=kk == 0, stop=kk == 3)
                lg = sb.tile([R, P], f32, tag="lg")
                nc.vector.tensor_scalar(out=lg[:], in0=ps2[:], scalar1=b2t[:],
                                        scalar2=None, op0=mybir.AluOpType.add)
                lt = psm.tile([P, R], f32, tag="lt")
                nc.tensor.transpose(out=lt[:], in_=lg[:], identity=ident[:R, :R])
                lts = sb.tile([P, R], f32, tag="lts")
                nc.vector.tensor_copy(lts[:], lt[:])
                nc.sync.dma_start(
                    out.rearrange("(t p) r -> p t r", p=P)[:, t, :], lts[:])

    nc.compile()
    return nc


def kernel(embed, temp, attn_w, attn_b, W1, b1, W2, b2,
           edge_index, H_idx, H_seg, T_idx, T_seg, B):
    embed = np.asarray(embed, np.float32)
    temp = np.asarray(temp, np.float32)
    attn_w = np.asarray(attn_w, np.float32)
    W1 = np.asarray(W1, np.float32)
    b1 = np.asarray(b1, np.float32)
    W2 = np.asarray(W2, np.float32)
    b2 = np.asarray(b2, np.float32)
    edge_index = np.asarray(edge_index)
    H_idx, H_seg = np.asarray(H_idx), np.asarray(H_seg)
    T_idx, T_seg = np.asarray(T_idx), np.asarray(T_seg)

    src, dst = edge_index[0].astype(np.int64), edge_index[1].astype(np.int64)

    # edges by dst shard; block/local ids within shard
    blkH, locH, srcH = [], [], []
    for c in range(NCORES):
        lo = c * SHARD
        m = (dst >= lo) & (dst < lo + SHARD)
        dl = dst[m] - lo
        blkH.append(dl >> 7)
        locH.append(dl & 127)
        srcH.append(src[m])
    gszH, goffH, tokH, hsrcs, hlocs = _schedule2(blkH, locH, srcH, NBLK, OCT)

    def pool_prep(idx, seg):
        blks, locs, idxs = [], [], []
        for c in range(NCORES):
            lo = c * BSEG
            m = (seg >= lo) & (seg < lo + BSEG)
            sl = seg[m].astype(np.int64) - lo
            blks.append(sl >> 7)
            locs.append(sl & 127)
            idxs.append(idx[m].astype(np.int64))
        return _schedule2(blks, locs, idxs, NSB, POCT)

    gszPh, goffPh, tokPh, psrcsH, plocsH = pool_prep(H_idx, H_seg.astype(np.int64))
    gszPt, goffPt, tokPt, psrcsT, plocsT = pool_prep(T_idx, T_seg.astype(np.int64))

    key = (tokH, tokPh, tokPt)
    if key not in _COMPILED:
        _COMPILED[key] = _build_program(gszH, goffH, tokH, gszPh, goffPh, tokPh,
                                        gszPt, goffPt, tokPt)
    nc = _COMPILED[key]

    dst_rp = np.zeros(N + 1, np.int64)
    dst_rp[1:] = np.cumsum(np.bincount(dst, minlength=N))
    src_rp = np.zeros(N + 1, np.int64)
    src_rp[1:] = np.cumsum(np.bincount(src, minlength=N))

    def shard_ptr(rp, c):
        lo = c * SHARD
        seg = np.zeros(SHARD + 1, np.int64)
        n_real = max(0, min(SHARD, N - lo))
        if n_real > 0:
            seg[:n_real + 1] = rp[lo:lo + n_real + 1]
            seg[n_real + 1:] = rp[lo + n_real]
        lo_a = seg[:-1].reshape(NBLK, P).T.astype(np.int32).copy()
        hi_a = seg[1:].reshape(NBLK, P).T.astype(np.int32).copy()
        return lo_a, hi_a

    def colmajor(arr, width):
        return np.ascontiguousarray(arr.reshape(len(arr) // P, P).T)

    bf = ml_dtypes.bfloat16
    in_maps = []
    for c in range(NCORES):
        lo = c * SHARD
        n_real = max(0, min(SHARD, N - lo))
        esh = np.zeros((SHARD, D), np.float32)
        esh[:n_real] = embed[lo:lo + n_real]
        dplc, dphc = shard_ptr(dst_rp, c)
        splc, sphc = shard_ptr(src_rp, c)
        in_maps.append(dict(
            embed_sh=esh,
            temp_in=np.tile(temp[None, :], (P, 1)),
            wrep_in=np.tile(attn_w[:, 0][None, :], (P, 1)),
            w1_in=W1.reshape(3, P, H_MLP).astype(bf),
            b1_in=np.ascontiguousarray(b1.reshape(4, P).T),
            w2_in=W2.reshape(4, P, R).astype(bf),
            b2_in=b2[:, None].copy(),
            dpl=dplc, dph=dphc, spl=splc, sph=sphc,
            hsrc=_wrap_idx16(hsrcs[c]),
            hloc=colmajor(hlocs[c], P),
            psrcH=_wrap_idx16(psrcsH[c]),
            plocH=colmajor(plocsH[c], P),
            psrcT=_wrap_idx16(psrcsT[c]),
            plocT=colmajor(plocsT[c], P),
        ))

    res = run_bass_kernel_spmd(nc, in_maps, list(range(NCORES)))
    return np.concatenate([res.results[c]["out"] for c in range(NCORES)], axis=0)

